# revision 19
# baseline (speedup 1.0000x reference)
"""Causal multi-head attention (B=1, S=2048, H=16, D=128, fp32) on 8 TRN2
NeuronCores - head parallelism (2 heads/core), no collectives.

v2 redesign vs the 62us baseline:
  - tri-masking via PE "ramp" matmul: accumulate -2000*max(0, t-s) into the
    diagonal psum strips with one extra N=256 matmul per (head, diag-group)
    (lhsT = incl-diag upper tri, rhs = strict lower tri * -2000).  exp() then
    produces exact fp16 zeros above the diagonal.  Kills all GpSimd masks.
  - softmax denominator: per-group pair-sums (expt0+expt1, one fp16 DVE add)
    are DMA'd to DRAM; the host does the 128-row partition reduction.  Kills
    the es accumulation adds (was ~35us of DVE).
  - exp split per group between ACT (flat cols [0:768], includes all ramp
    regions) and DVE Schraudolph ([768:1024], real scores only).
  - psum_o -> sbuf output copies on GpSimd (otherwise idle), output fp16.
  - 20 warmup matmuls (HAM clock warm) instead of 40; input DMA reordered so
    the first real matmul can start ~4us earlier.
"""

import math

import numpy as np

import concourse.mybir as mybir
import concourse.tile as tile
from concourse import bacc
from concourse.masks import make_lower_triangular, make_upper_triangular

S = 2048
H = 16
D = 128
HC = 2  # heads per core
NCORES = 8
P = 128
SBLK = 512  # s-block width
NT = S // P  # 16 t tiles
NB = S // SBLK  # 4 s blocks / chunks
TPB = SBLK // P  # 4 t tiles per s block
SCALE = 1.0 / math.sqrt(D)
LOG2E = math.log2(math.e)

# Schraudolph fp16 exp on DVE: bits = in*A + B converted to int16
SCHRAU_C = 59.0
A_DVE = 1024.0 * LOG2E * SCALE
B_DVE = 15.0 * 1024.0 - SCHRAU_C

F32 = mybir.dt.float32
F16 = mybir.dt.float16
I16 = mybir.dt.int16

MULT = mybir.AluOpType.mult
ADD = mybir.AluOpType.add

ACT_W = 768  # flat exp columns [0:ACT_W] on ACT, [ACT_W:1024] on DVE
RAMP_VAL = -2000.0
N_WARM = 27

# groups of two t tiles sharing one psum tile: (i0, i1, s_lo0, s_lo1, is_diag)
BLOCK_GROUPS = {}
for _b in range(NB):
    _n = TPB * _b
    _gs = [(ip, ip + 1, 0, 0, False) for ip in range(0, _n, 2)]
    _gs += [
        (_n, _n + 1, 0, P, True),
        (_n + 2, _n + 3, 2 * P, 3 * P, True),
    ]
    BLOCK_GROUPS[_b] = _gs

TASKS = [(b, g) for b in range(NB) for g in range(len(BLOCK_GROUPS[b]))]
NG = len(TASKS)  # 20


def pair_valid_start(b, g):
    """first valid column of the pair-sum tile for group (b, g)"""
    i0, i1, s0, s1, is_diag = BLOCK_GROUPS[b][g]
    return s0  # 0 for full and dg0, 2P for dg1


def build_nc():
    nc = bacc.Bacc("TRN2", target_bir_lowering=False, debug=False, num_devices=NCORES)
    qk_d = nc.dram_tensor("qk", [NB, D, 2, HC, SBLK], F16, kind="ExternalInput").ap()
    v_d = nc.dram_tensor("v", [NB, P, TPB, HC, D], F16, kind="ExternalInput").ap()
    ot_d = nc.dram_tensor("ot", [NB, P, HC, SBLK], F16, kind="ExternalOutput").ap()
    pr_d = nc.dram_tensor("pr", [NG, P, HC, SBLK], F16, kind="ExternalOutput").ap()

    with tile.TileContext(nc) as tc:
        with (
            tc.tile_pool(name="consts", bufs=1) as cpool,
            tc.tile_pool(name="big", bufs=1) as bigpool,
            tc.tile_pool(name="exp", bufs=8) as epool,
            tc.tile_pool(name="pairs", bufs=6) as prpool,
            tc.tile_pool(name="otn", bufs=2) as opool,
            tc.tile_pool(name="psum_s", bufs=2, space="PSUM") as ps_pool,
            tc.tile_pool(name="psum_o", bufs=3, space="PSUM") as po_pool,
            tc.tile_pool(name="psum_w", bufs=1, space="PSUM") as pw_pool,
        ):
            # constants: warmup weights (DVE memset, ready immediately) and
            # the two triangular ramp matrices (GpSimd)
            wconst = cpool.tile([P, P], F16, tag="wconst")
            nc.vector.memset(wconst, 0.25)
            m1 = cpool.tile([P, P], F16, tag="m1")
            make_upper_triangular(nc, m1, val=1.0, diag=True)
            m2 = cpool.tile([P, 2, P], F16, tag="m2")
            for j in range(2):
                make_lower_triangular(nc, m2[:, j, :], val=RAMP_VAL, diag=False)

            # PE warmup while input DMAs stream (HAM clock warm-up)
            warm_ps = pw_pool.tile([P, P], F32, tag="warm")
            for _ in range(N_WARM):
                nc.tensor.matmul(
                    warm_ps[:],
                    wconst[:],
                    wconst[:],
                    start=True,
                    stop=True,
                    skip_group_check=True,
                )

            # chunked SBUF inputs, loaded in consumption order
            qk_c = {}
            vb_c = {}
            for c in range(NB):
                qk_c[c] = bigpool.tile(
                    [P, 2, HC, SBLK], F16, tag=f"qkc{c}", name=f"qkc{c}"
                )
                vb_c[c] = bigpool.tile(
                    [P, TPB, HC, D], F16, tag=f"vbc{c}", name=f"vbc{c}"
                )
            # whole-chunk contiguous DMAs (4KB/partition rows -> large
            # packets); early DMA is bandwidth-bound, not issue-bound.
            for c in range(NB):
                nc.sync.dma_start(qk_c[c][:], qk_d[c])
                nc.sync.dma_start(vb_c[c][:], v_d[c])

            def kt_tile(h, i):
                return qk_c[i // TPB][:, 1, h, (i % TPB) * P : (i % TPB + 1) * P]

            def qt_block(h, b):
                return qk_c[b][:, 0, h, :]

            def v_tile(h, i):
                return vb_c[i // TPB][:, i % TPB, h, :]

            psum_o = {}
            expt_of = {}

            def emit_mm1_exp(b, g, h):
                i0, i1, s0, s1, is_diag = BLOCK_GROUPS[b][g]
                ps = ps_pool.tile([P, 2, SBLK], F32, tag="ps", name=f"ps{h}_{b}_{g}")
                ex = epool.tile([P, 2, SBLK], F16, tag="ex", name=f"ex{h}_{b}_{g}")
                psf = ps.rearrange("p a s -> p (a s)")
                exf = ex.rearrange("p a s -> p (a s)")
                if not is_diag:
                    for j, i in enumerate((i0, i1)):
                        nc.tensor.matmul(
                            ps[:, j, :],
                            kt_tile(h, i),
                            qt_block(h, b),
                            start=True,
                            stop=True,
                        )
                else:
                    for j, (i, s_lo) in enumerate(((i0, s0), (i1, s1))):
                        nc.tensor.matmul(
                            ps[:, j, s_lo:],
                            kt_tile(h, i),
                            qt_block(h, b)[:, s_lo:],
                            start=True,
                            stop=False,
                        )
                    # ramp over each diagonal strip:
                    # psum[t, j, sj:sj+P] += -2000 * max(0, t - s)
                    for j, s_lo in enumerate((s0, s1)):
                        nc.tensor.matmul(
                            ps[:, j, s_lo : s_lo + P],
                            m1[:],
                            m2[:, j, :],
                            start=False,
                            stop=True,
                            skip_group_check=True,
                        )
                # exp: ACT takes the flat head [0:ACT_W] (includes all ramp
                # regions -> exact zeros), DVE Schraudolph takes the tail
                if not is_diag or s0 == 0:
                    nc.scalar.activation(
                        exf[:, :ACT_W],
                        psf[:, :ACT_W],
                        mybir.ActivationFunctionType.Exp,
                        scale=SCALE,
                    )
                    nc.vector.tensor_scalar(
                        exf[:, ACT_W:].bitcast(I16),
                        psf[:, ACT_W:],
                        A_DVE,
                        B_DVE,
                        MULT,
                        ADD,
                    )
                else:
                    # dg1: valid = t0[2P:], t1[3P:].  ACT does the two ramped
                    # strips {t0[2P:3P], t1[3P:]}, DVE does t0[3P:].
                    for j, s_lo in ((0, s0), (1, s1)):
                        nc.scalar.activation(
                            ex[:, j, s_lo : s_lo + P],
                            ps[:, j, s_lo : s_lo + P],
                            mybir.ActivationFunctionType.Exp,
                            scale=SCALE,
                        )
                    nc.vector.tensor_scalar(
                        exf[:, 3 * P : SBLK].bitcast(I16),
                        psf[:, 3 * P : SBLK],
                        A_DVE,
                        B_DVE,
                        MULT,
                        ADD,
                    )
                expt_of[h, b, g] = ex

            def emit_mm2(b, g, h):
                i0, i1, s0, s1, is_diag = BLOCK_GROUPS[b][g]
                last_i = TPB * b + TPB - 1
                ex = expt_of[h, b, g]
                if (h, b) not in psum_o:
                    psum_o[h, b] = po_pool.tile(
                        [P, SBLK], F32, tag="po", name=f"po{h}_{b}"
                    )
                for j, (i, s_lo) in enumerate(((i0, s0), (i1, s1))):
                    nc.tensor.matmul(
                        psum_o[h, b][:, s_lo:],
                        v_tile(h, i),
                        ex[:, j, s_lo:],
                        start=(i == 0),
                        stop=(i == last_i),
                        skip_group_check=True,
                    )

            def emit_pair_dma(b, g, gidx, eng=None):
                i0, i1, s0, s1, is_diag = BLOCK_GROUPS[b][g]
                pr = prpool.tile([P, HC, SBLK], F16, tag="pr", name=f"pr{gidx}")
                for h in range(HC):
                    ex = expt_of.pop((h, b, g))
                    exf = ex.rearrange("p a s -> p (a s)")
                    if not is_diag:
                        nc.vector.tensor_add(
                            out=pr[:, h, :],
                            in0=exf[:, :SBLK],
                            in1=exf[:, SBLK:],
                        )
                    else:
                        # t1 has no valid columns below s1; t0 alone covers
                        # [s0:s1) (its masked part is exact zeros)
                        nc.vector.tensor_copy(
                            out=pr[:, h, s0:s1], in_=ex[:, 0, s0:s1]
                        )
                        nc.vector.tensor_add(
                            out=pr[:, h, s1:],
                            in0=ex[:, 0, s1:],
                            in1=ex[:, 1, s1:],
                        )
                (eng or nc.sync).dma_start(pr_d[gidx, :, :, s0:], pr[:, :, s0:])

            otn_of = {}

            def emit_copies(b, lo=0, hi=SBLK, eng=None):
                if b not in otn_of:
                    otn_of[b] = opool.tile(
                        [P, HC, SBLK], F16, tag="otn", name=f"otn{b}"
                    )
                otn = otn_of[b]
                nc.scalar.copy(otn[:, 0, lo:hi], psum_o[0, b][:, lo:hi])
                nc.vector.tensor_copy(
                    out=otn[:, 1, lo:hi], in_=psum_o[1, b][:, lo:hi]
                )
                if hi == SBLK:
                    psum_o.pop((0, b))
                    psum_o.pop((1, b))
                (eng or nc.sync).dma_start(ot_d[b, :, :, lo:hi], otn[:, :, lo:hi])

            gidx_of = {t: i for i, t in enumerate(TASKS)}
            pend = []
            copies_due = {}
            for k, (b, g) in enumerate(TASKS):
                for h in range(HC):
                    emit_mm1_exp(b, g, h)
                for blk in copies_due.pop(k, []):
                    emit_copies(blk)
                if pend:
                    pb, pg = pend[-1]
                    for h in range(HC):
                        emit_mm2(pb, pg, h)
                    if pg == len(BLOCK_GROUPS[pb]) - 1:
                        copies_due.setdefault(k + 1, []).append(pb)
                    elif (pb, pg) == (NB - 1, len(BLOCK_GROUPS[NB - 1]) - 2):
                        # block 3 cols [0:2P] are final after dg0's mm2 --
                        # ship them early to shorten the tail (on the scalar
                        # HWDGE queue, which is idle at the tail)
                        emit_copies(NB - 1, 0, 2 * P, eng=nc.scalar)
                if len(pend) == 2:
                    db, dg = pend.pop(0)
                    emit_pair_dma(db, dg, gidx_of[(db, dg)])
                pend.append((b, g))
            # flush: pend = [(3, G-2), (3, G-1)]
            db, dg = pend[0]
            emit_pair_dma(db, dg, gidx_of[(db, dg)], eng=nc.scalar)
            pb, pg = pend[-1]
            for h in range(HC):
                emit_mm2(pb, pg, h)
            emit_pair_dma(pb, pg, gidx_of[(pb, pg)], eng=nc.scalar)
            for blks in copies_due.values():
                for blk in blks:
                    emit_copies(blk)
            emit_copies(NB - 1, 2 * P, SBLK, eng=nc.scalar)
    nc.compile()
    return nc


_NC_CACHE = None


def _get_nc():
    global _NC_CACHE
    if _NC_CACHE is None:
        _NC_CACHE = build_nc()
    return _NC_CACHE


def make_in_maps(query, key, value):
    query = np.asarray(query)
    key = np.asarray(key)
    value = np.asarray(value)
    in_maps = []
    for c in range(NCORES):
        hs = slice(c * HC, (c + 1) * HC)
        # [D, HC, S] views of this core's heads
        qD = query[0, :, hs, :].transpose(2, 1, 0)
        kD = key[0, :, hs, :].transpose(2, 1, 0)
        qk = np.empty((NB, D, 2, HC, SBLK), np.float16)
        for cc in range(NB):
            cs = slice(cc * SBLK, (cc + 1) * SBLK)
            qk[cc, :, 0] = qD[:, :, cs]
            qk[cc, :, 1] = kD[:, :, cs]
        v5 = (
            value[0][:, hs, :]
            .reshape(NB, TPB, P, HC, D)
            .transpose(0, 2, 1, 3, 4)
        )
        in_maps.append(
            {
                "qk": qk,
                "v": np.ascontiguousarray(v5).astype(np.float16),
            }
        )
    return in_maps


def kernel(query, key, value):
    from concourse.bass_utils import run_bass_kernel_spmd

    nc = _get_nc()
    in_maps = make_in_maps(query, key, value)
    res = run_bass_kernel_spmd(nc, in_maps, core_ids=list(range(NCORES)))
    out = np.empty((1, S, H, D), dtype=np.float32)
    for c in range(NCORES):
        ot = res.results[c]["ot"].astype(np.float32)  # [NB, D, HC, SBLK]
        pr = res.results[c]["pr"]  # [NG, P, HC, SBLK] f16 pair sums
        l = np.zeros((HC, S), np.float32)
        for gidx, (b, g) in enumerate(TASKS):
            s0 = pair_valid_start(b, g)
            seg = pr[gidx, :, :, s0:].astype(np.float32).sum(axis=0)  # [HC, w]
            l[:, b * SBLK + s0 : (b + 1) * SBLK] += seg
        for b in range(NB):
            seg = ot[b] / l[:, b * SBLK : (b + 1) * SBLK][None, :, :]
            out[0, b * SBLK : (b + 1) * SBLK, c * HC : (c + 1) * HC, :] = (
                seg.transpose(2, 1, 0)
            )
    return out


# revision 20
# speedup vs baseline: 1.0385x; 1.0385x over previous
"""Causal multi-head attention (B=1, S=2048, H=16, D=128, fp32) on 8 TRN2
NeuronCores - head parallelism (2 heads/core), no collectives.

v2 redesign vs the 62us baseline:
  - tri-masking via PE "ramp" matmul: accumulate -2000*max(0, t-s) into the
    diagonal psum strips with one extra N=256 matmul per (head, diag-group)
    (lhsT = incl-diag upper tri, rhs = strict lower tri * -2000).  exp() then
    produces exact fp16 zeros above the diagonal.  Kills all GpSimd masks.
  - softmax denominator: per-group pair-sums (expt0+expt1, one fp16 DVE add)
    are DMA'd to DRAM; the host does the 128-row partition reduction.  Kills
    the es accumulation adds (was ~35us of DVE).
  - exp split per group between ACT (flat cols [0:768], includes all ramp
    regions) and DVE Schraudolph ([768:1024], real scores only).
  - psum_o -> sbuf output copies on GpSimd (otherwise idle), output fp16.
  - 20 warmup matmuls (HAM clock warm) instead of 40; input DMA reordered so
    the first real matmul can start ~4us earlier.
"""

import math

import numpy as np

import concourse.mybir as mybir
import concourse.tile as tile
from concourse import bacc
from concourse.masks import make_lower_triangular, make_upper_triangular

S = 2048
H = 16
D = 128
HC = 2  # heads per core
NCORES = 8
P = 128
SBLK = 512  # s-block width
NT = S // P  # 16 t tiles
NB = S // SBLK  # 4 s blocks / chunks
TPB = SBLK // P  # 4 t tiles per s block
SCALE = 1.0 / math.sqrt(D)
LOG2E = math.log2(math.e)

# Schraudolph fp16 exp on DVE: bits = in*A + B converted to int16
SCHRAU_C = 59.0
A_DVE = 1024.0 * LOG2E * SCALE
B_DVE = 15.0 * 1024.0 - SCHRAU_C

F32 = mybir.dt.float32
F16 = mybir.dt.float16
I16 = mybir.dt.int16

MULT = mybir.AluOpType.mult
ADD = mybir.AluOpType.add

ACT_W = 768  # flat exp columns [0:ACT_W] on ACT, [ACT_W:1024] on DVE
RAMP_VAL = -2000.0
N_WARM = 50

# groups of two t tiles sharing one psum tile: (i0, i1, s_lo0, s_lo1, is_diag)
BLOCK_GROUPS = {}
for _b in range(NB):
    _n = TPB * _b
    _gs = [(ip, ip + 1, 0, 0, False) for ip in range(0, _n, 2)]
    _gs += [
        (_n, _n + 1, 0, P, True),
        (_n + 2, _n + 3, 2 * P, 3 * P, True),
    ]
    BLOCK_GROUPS[_b] = _gs

TASKS = [(b, g) for b in range(NB) for g in range(len(BLOCK_GROUPS[b]))]
NG = len(TASKS)  # 20


def pair_valid_start(b, g):
    """first valid column of the pair-sum tile for group (b, g)"""
    i0, i1, s0, s1, is_diag = BLOCK_GROUPS[b][g]
    return s0  # 0 for full and dg0, 2P for dg1


def build_nc():
    nc = bacc.Bacc("TRN2", target_bir_lowering=False, debug=False, num_devices=NCORES)
    qk_d = nc.dram_tensor("qk", [NB, D, 2, HC, SBLK], F16, kind="ExternalInput").ap()
    v_d = nc.dram_tensor("v", [NB, P, TPB, HC, D], F16, kind="ExternalInput").ap()
    ot_d = nc.dram_tensor("ot", [NB, P, HC, SBLK], F16, kind="ExternalOutput").ap()
    pr_d = nc.dram_tensor("pr", [NG, P, HC, SBLK], F16, kind="ExternalOutput").ap()

    with tile.TileContext(nc) as tc:
        with (
            tc.tile_pool(name="consts", bufs=1) as cpool,
            tc.tile_pool(name="big", bufs=1) as bigpool,
            tc.tile_pool(name="exp", bufs=8) as epool,
            tc.tile_pool(name="pairs", bufs=6) as prpool,
            tc.tile_pool(name="otn", bufs=2) as opool,
            tc.tile_pool(name="psum_s", bufs=2, space="PSUM") as ps_pool,
            tc.tile_pool(name="psum_o", bufs=3, space="PSUM") as po_pool,
            tc.tile_pool(name="psum_w", bufs=1, space="PSUM") as pw_pool,
        ):
            # constants: warmup weights (DVE memset, ready immediately) and
            # the two triangular ramp matrices (GpSimd)
            wconst = cpool.tile([P, P], F16, tag="wconst")
            nc.vector.memset(wconst, 0.25)
            m1 = cpool.tile([P, P], F16, tag="m1")
            make_upper_triangular(nc, m1, val=1.0, diag=True)
            m2 = cpool.tile([P, 2, P], F16, tag="m2")
            for j in range(2):
                make_lower_triangular(nc, m2[:, j, :], val=RAMP_VAL, diag=False)

            # PE warmup while input DMAs stream (HAM clock warm-up)
            warm_ps = pw_pool.tile([P, P], F32, tag="warm")
            for _ in range(N_WARM):
                nc.tensor.matmul(
                    warm_ps[:],
                    wconst[:],
                    wconst[:],
                    start=True,
                    stop=True,
                    skip_group_check=True,
                )

            # chunked SBUF inputs, loaded in consumption order
            qk_c = {}
            vb_c = {}
            for c in range(NB):
                qk_c[c] = bigpool.tile(
                    [P, 2, HC, SBLK], F16, tag=f"qkc{c}", name=f"qkc{c}"
                )
                vb_c[c] = bigpool.tile(
                    [P, TPB, HC, D], F16, tag=f"vbc{c}", name=f"vbc{c}"
                )
            # whole-chunk contiguous DMAs (4KB/partition rows -> large
            # packets); early DMA is bandwidth-bound, not issue-bound.
            for c in range(NB):
                nc.sync.dma_start(qk_c[c][:], qk_d[c])
                nc.sync.dma_start(vb_c[c][:], v_d[c])

            def kt_tile(h, i):
                return qk_c[i // TPB][:, 1, h, (i % TPB) * P : (i % TPB + 1) * P]

            def qt_block(h, b):
                return qk_c[b][:, 0, h, :]

            def v_tile(h, i):
                return vb_c[i // TPB][:, i % TPB, h, :]

            psum_o = {}
            expt_of = {}

            def emit_mm1_exp(b, g, h):
                i0, i1, s0, s1, is_diag = BLOCK_GROUPS[b][g]
                ps = ps_pool.tile([P, 2, SBLK], F32, tag="ps", name=f"ps{h}_{b}_{g}")
                ex = epool.tile([P, 2, SBLK], F16, tag="ex", name=f"ex{h}_{b}_{g}")
                psf = ps.rearrange("p a s -> p (a s)")
                exf = ex.rearrange("p a s -> p (a s)")
                if not is_diag:
                    for j, i in enumerate((i0, i1)):
                        nc.tensor.matmul(
                            ps[:, j, :],
                            kt_tile(h, i),
                            qt_block(h, b),
                            start=True,
                            stop=True,
                        )
                else:
                    for j, (i, s_lo) in enumerate(((i0, s0), (i1, s1))):
                        nc.tensor.matmul(
                            ps[:, j, s_lo:],
                            kt_tile(h, i),
                            qt_block(h, b)[:, s_lo:],
                            start=True,
                            stop=False,
                        )
                    # ramp over each diagonal strip:
                    # psum[t, j, sj:sj+P] += -2000 * max(0, t - s)
                    for j, s_lo in enumerate((s0, s1)):
                        nc.tensor.matmul(
                            ps[:, j, s_lo : s_lo + P],
                            m1[:],
                            m2[:, j, :],
                            start=False,
                            stop=True,
                            skip_group_check=True,
                        )
                # exp: ACT takes the flat head [0:ACT_W] (includes all ramp
                # regions -> exact zeros), DVE Schraudolph takes the tail
                if not is_diag or s0 == 0:
                    nc.scalar.activation(
                        exf[:, :ACT_W],
                        psf[:, :ACT_W],
                        mybir.ActivationFunctionType.Exp,
                        scale=SCALE,
                    )
                    nc.vector.tensor_scalar(
                        exf[:, ACT_W:].bitcast(I16),
                        psf[:, ACT_W:],
                        A_DVE,
                        B_DVE,
                        MULT,
                        ADD,
                    )
                else:
                    # dg1: valid = t0[2P:], t1[3P:].  ACT does the two ramped
                    # strips {t0[2P:3P], t1[3P:]}, DVE does t0[3P:].
                    for j, s_lo in ((0, s0), (1, s1)):
                        nc.scalar.activation(
                            ex[:, j, s_lo : s_lo + P],
                            ps[:, j, s_lo : s_lo + P],
                            mybir.ActivationFunctionType.Exp,
                            scale=SCALE,
                        )
                    nc.vector.tensor_scalar(
                        exf[:, 3 * P : SBLK].bitcast(I16),
                        psf[:, 3 * P : SBLK],
                        A_DVE,
                        B_DVE,
                        MULT,
                        ADD,
                    )
                expt_of[h, b, g] = ex

            def emit_mm2(b, g, h):
                i0, i1, s0, s1, is_diag = BLOCK_GROUPS[b][g]
                last_i = TPB * b + TPB - 1
                ex = expt_of[h, b, g]
                if (h, b) not in psum_o:
                    psum_o[h, b] = po_pool.tile(
                        [P, SBLK], F32, tag="po", name=f"po{h}_{b}"
                    )
                for j, (i, s_lo) in enumerate(((i0, s0), (i1, s1))):
                    nc.tensor.matmul(
                        psum_o[h, b][:, s_lo:],
                        v_tile(h, i),
                        ex[:, j, s_lo:],
                        start=(i == 0),
                        stop=(i == last_i),
                        skip_group_check=True,
                    )

            def emit_pair_dma(b, g, gidx, eng=None):
                i0, i1, s0, s1, is_diag = BLOCK_GROUPS[b][g]
                pr = prpool.tile([P, HC, SBLK], F16, tag="pr", name=f"pr{gidx}")
                for h in range(HC):
                    ex = expt_of.pop((h, b, g))
                    exf = ex.rearrange("p a s -> p (a s)")
                    if not is_diag:
                        nc.vector.tensor_add(
                            out=pr[:, h, :],
                            in0=exf[:, :SBLK],
                            in1=exf[:, SBLK:],
                        )
                    else:
                        # t1 has no valid columns below s1; t0 alone covers
                        # [s0:s1) (its masked part is exact zeros)
                        nc.vector.tensor_copy(
                            out=pr[:, h, s0:s1], in_=ex[:, 0, s0:s1]
                        )
                        nc.vector.tensor_add(
                            out=pr[:, h, s1:],
                            in0=ex[:, 0, s1:],
                            in1=ex[:, 1, s1:],
                        )
                (eng or nc.sync).dma_start(pr_d[gidx, :, :, s0:], pr[:, :, s0:])

            otn_of = {}

            def emit_copies(b, lo=0, hi=SBLK, eng=None):
                if b not in otn_of:
                    otn_of[b] = opool.tile(
                        [P, HC, SBLK], F16, tag="otn", name=f"otn{b}"
                    )
                otn = otn_of[b]
                nc.scalar.copy(otn[:, 0, lo:hi], psum_o[0, b][:, lo:hi])
                nc.vector.tensor_copy(
                    out=otn[:, 1, lo:hi], in_=psum_o[1, b][:, lo:hi]
                )
                if hi == SBLK:
                    psum_o.pop((0, b))
                    psum_o.pop((1, b))
                (eng or nc.sync).dma_start(ot_d[b, :, :, lo:hi], otn[:, :, lo:hi])

            gidx_of = {t: i for i, t in enumerate(TASKS)}
            pend = []
            copies_due = {}
            for k, (b, g) in enumerate(TASKS):
                for h in range(HC):
                    emit_mm1_exp(b, g, h)
                for blk in copies_due.pop(k, []):
                    emit_copies(blk)
                if pend:
                    pb, pg = pend[-1]
                    for h in range(HC):
                        emit_mm2(pb, pg, h)
                    if pg == len(BLOCK_GROUPS[pb]) - 1:
                        copies_due.setdefault(k + 1, []).append(pb)
                    elif (pb, pg) == (NB - 1, len(BLOCK_GROUPS[NB - 1]) - 2):
                        # block 3 cols [0:2P] are final after dg0's mm2 --
                        # ship them early to shorten the tail (on the scalar
                        # HWDGE queue, which is idle at the tail)
                        emit_copies(NB - 1, 0, 2 * P, eng=nc.scalar)
                if len(pend) == 2:
                    db, dg = pend.pop(0)
                    emit_pair_dma(db, dg, gidx_of[(db, dg)])
                pend.append((b, g))
            # flush: pend = [(3, G-2), (3, G-1)]
            db, dg = pend[0]
            emit_pair_dma(db, dg, gidx_of[(db, dg)], eng=nc.scalar)
            pb, pg = pend[-1]
            for h in range(HC):
                emit_mm2(pb, pg, h)
            emit_pair_dma(pb, pg, gidx_of[(pb, pg)], eng=nc.scalar)
            for blks in copies_due.values():
                for blk in blks:
                    emit_copies(blk)
            emit_copies(NB - 1, 2 * P, SBLK, eng=nc.scalar)
    nc.compile()
    return nc


_NC_CACHE = None


def _get_nc():
    global _NC_CACHE
    if _NC_CACHE is None:
        _NC_CACHE = build_nc()
    return _NC_CACHE


def make_in_maps(query, key, value):
    query = np.asarray(query)
    key = np.asarray(key)
    value = np.asarray(value)
    in_maps = []
    for c in range(NCORES):
        hs = slice(c * HC, (c + 1) * HC)
        # [D, HC, S] views of this core's heads
        qD = query[0, :, hs, :].transpose(2, 1, 0)
        kD = key[0, :, hs, :].transpose(2, 1, 0)
        qk = np.empty((NB, D, 2, HC, SBLK), np.float16)
        for cc in range(NB):
            cs = slice(cc * SBLK, (cc + 1) * SBLK)
            qk[cc, :, 0] = qD[:, :, cs]
            qk[cc, :, 1] = kD[:, :, cs]
        v5 = (
            value[0][:, hs, :]
            .reshape(NB, TPB, P, HC, D)
            .transpose(0, 2, 1, 3, 4)
        )
        in_maps.append(
            {
                "qk": qk,
                "v": np.ascontiguousarray(v5).astype(np.float16),
            }
        )
    return in_maps


def kernel(query, key, value):
    from concourse.bass_utils import run_bass_kernel_spmd

    nc = _get_nc()
    in_maps = make_in_maps(query, key, value)
    res = run_bass_kernel_spmd(nc, in_maps, core_ids=list(range(NCORES)))
    out = np.empty((1, S, H, D), dtype=np.float32)
    for c in range(NCORES):
        ot = res.results[c]["ot"].astype(np.float32)  # [NB, D, HC, SBLK]
        pr = res.results[c]["pr"]  # [NG, P, HC, SBLK] f16 pair sums
        l = np.zeros((HC, S), np.float32)
        for gidx, (b, g) in enumerate(TASKS):
            s0 = pair_valid_start(b, g)
            seg = pr[gidx, :, :, s0:].astype(np.float32).sum(axis=0)  # [HC, w]
            l[:, b * SBLK + s0 : (b + 1) * SBLK] += seg
        for b in range(NB):
            seg = ot[b] / l[:, b * SBLK : (b + 1) * SBLK][None, :, :]
            out[0, b * SBLK : (b + 1) * SBLK, c * HC : (c + 1) * HC, :] = (
                seg.transpose(2, 1, 0)
            )
    return out
